# revision 16
# baseline (speedup 1.0000x reference)
"""Multi-head attention (B=2, L=2048, E=1024, H=16) on 8 trn2 NeuronCores.

Sharding: core c -> (batch b = c//4, head-group g = c%4). Each core computes
4 heads (256 feature dims) of one batch: Q/K/V projections column-sliced,
full attention for its heads, and its slice of the output projection
(Wo row-parallel). Host sums the 4 partial products per batch and adds
bo + Wo@bv (the bv term commutes through softmax-normalized attention).

Device layout notes:
 - activations kept feature-on-partitions ("transposed"): qT/kT [256, L]
 - scores computed transposed: sT[k, q] = kT-slice.T @ qT-slice, so exp runs
   tile-local; the key-axis softmax sum comes free from a ones-column
   appended to V in the PV matmul (out row 64 = sum of exp).
 - softmax without max-subtraction: scores are O(1) here and masked entries
   are -1e9 -> exp underflows to exactly 0. Shift-invariance makes this
   mathematically identical to the reference.
 - matmuls run as float32r (full PE rate at moving-dim >= 256); fp32r inputs
   must be produced rounded, so DMA-loaded operands get a rounding copy.
"""

import numpy as np

B, L, E, H = 2, 2048, 1024, 16
Dh = 64
NCORES = 8
HPC = 4           # heads per core
DG = HPC * Dh     # 256 features per core
Dv = Dh + 1       # v width incl. ones column
NEG = np.float32(-1e9)

USE_F32R = True

_CACHE = {}
LAST_RESULTS = None


def _build(variant, Lb, use_f32r, reps=1):
    import concourse.bacc as bacc
    import concourse.tile as tile
    from concourse import mybir
    from contextlib import ExitStack

    f32 = mybir.dt.float32
    nq = Lb // 512         # q blocks
    nk = Lb // 128         # k blocks == l tiles
    ne = E // 128          # e-chunks (8)
    ndc = DG // 128        # feature chunks per core (2)

    nc = bacc.Bacc()
    xqT = nc.dram_tensor("xqT", [E, Lb], f32, kind="ExternalInput")
    xkT = nc.dram_tensor("xkT", [E, Lb], f32, kind="ExternalInput")
    xvT = nc.dram_tensor("xvT", [E, Lb], f32, kind="ExternalInput")
    wqT = nc.dram_tensor("wqT", [E, DG], f32, kind="ExternalInput")
    wkT = nc.dram_tensor("wkT", [E, DG], f32, kind="ExternalInput")
    wvT = nc.dram_tensor("wvT", [E, DG], f32, kind="ExternalInput")
    bqs = nc.dram_tensor("bqs", [128, ndc], f32, kind="ExternalInput")
    bks = nc.dram_tensor("bks", [128, ndc], f32, kind="ExternalInput")
    woT = nc.dram_tensor("woT", [Dh, HPC, E], f32, kind="ExternalInput")
    dmask = maskT = None
    if variant == "causal":
        dmask = nc.dram_tensor("dmask", [128, 4, 512], f32, kind="ExternalInput")
    elif variant == "general":
        maskT = nc.dram_tensor("maskT", [Lb, Lb], f32, kind="ExternalInput")
    yT = nc.dram_tensor("yT", [E, Lb], f32, kind="ExternalOutput")

    # fp32r tiles must be written by compute instructions (rounding).
    cdt = mybir.dt.float32r if use_f32r else f32
    Exp = mybir.ActivationFunctionType.Exp
    Ident = mybir.ActivationFunctionType.Identity
    Copy = mybir.ActivationFunctionType.Copy

    with tile.TileContext(nc) as tc, ExitStack() as ctx:
        persist = ctx.enter_context(tc.tile_pool(name="persist", bufs=1))
        qT_s = persist.tile([128, ndc, Lb], cdt, tag="qT")
        kT_s = persist.tile([128, ndc, Lb], cdt, tag="kT")
        v_s = persist.tile([128, nk, HPC, Dv], cdt, tag="v")
        wo_s = persist.tile([Dh, HPC, E], cdt, tag="wo")
        bq_s = persist.tile([128, ndc], f32, tag="bq")
        bk_s = persist.tile([128, ndc], f32, tag="bk")
        wo_l = (persist.tile([Dh, HPC, E], f32, tag="wo_l", name="wo_l")
                if use_f32r else None)
        dm_s = (persist.tile([128, 4, 512], f32, tag="dm", name="dm")
                if variant == "causal" else None)

        for _rep in range(reps):
            if use_f32r:
                nc.sync.dma_start(out=wo_l, in_=woT[:, :, :])
                nc.vector.tensor_copy(out=wo_s, in_=wo_l)
            else:
                nc.sync.dma_start(out=wo_s, in_=woT[:, :, :])
            nc.sync.dma_start(out=bq_s, in_=bqs[:, :])
            nc.sync.dma_start(out=bk_s, in_=bks[:, :])
            if variant == "causal":
                nc.sync.dma_start(out=dm_s, in_=dmask[:, :, :])
            nc.vector.memset(v_s[:, :, :, Dh:Dv].bitcast(f32), 1.0)

            # ---------------- projections ----------------
            # per tensor: keep all 8 e-chunks of x resident, loop output-block
            # outer / e-chunk inner so psum accumulation groups are sequential.
            with tc.tile_pool(name="wproj", bufs=1) as wpool, \
                 tc.tile_pool(name="wload", bufs=2) as wlp:

                def load_w(wT, wtag):
                    w_s = wpool.tile([128, ne, DG], cdt, tag=wtag, name=wtag)
                    for ec in range(ne):
                        if use_f32r:
                            wl = wlp.tile([128, DG], f32, tag="wl", name="wl")
                            nc.sync.dma_start(
                                out=wl, in_=wT[ec * 128:(ec + 1) * 128, :])
                            nc.scalar.copy(out=w_s[:, ec, :], in_=wl)
                        else:
                            nc.sync.dma_start(
                                out=w_s[:, ec, :],
                                in_=wT[ec * 128:(ec + 1) * 128, :])
                    return w_s

                def load_x(xr, xT, ec):
                    xt = xr.tile([128, Lb], cdt, tag=f"x{ec}", name=f"x{ec}")
                    if use_f32r:
                        xl = xr.tile([128, Lb], f32, tag="xl", name="xl", bufs=2)
                        nc.sync.dma_start(
                            out=xl, in_=xT[ec * 128:(ec + 1) * 128, :])
                        nc.vector.tensor_copy(out=xt, in_=xl)
                    else:
                        nc.sync.dma_start(
                            out=xt, in_=xT[ec * 128:(ec + 1) * 128, :])
                    return xt

                for name, xT, wT, wtag, bias_t, scale, outT in (
                    ("q", xqT, wqT, "wq", bq_s, 0.125, qT_s),
                    ("k", xkT, wkT, "wk", bk_s, 1.0, kT_s),
                ):
                    w_s = load_w(wT, wtag)
                    with tc.tile_pool(name=f"x_{name}", bufs=1) as xr, \
                         tc.tile_pool(name=f"ps_{name}", bufs=4,
                                      space="PSUM") as pp:
                        xts = [load_x(xr, xT, ec) for ec in range(ne)]
                        for dc in range(ndc):
                            for ln in range(nq):
                                ps = pp.tile([128, 512], f32, tag="pj",
                                             name="pj")
                                for ec in range(ne):
                                    nc.tensor.matmul(
                                        out=ps,
                                        lhsT=w_s[:, ec, dc * 128:(dc + 1) * 128],
                                        rhs=xts[ec][:, ln * 512:(ln + 1) * 512],
                                        start=(ec == 0), stop=(ec == ne - 1),
                                    )
                                nc.scalar.activation(
                                    out=outT[:, dc, ln * 512:(ln + 1) * 512],
                                    in_=ps,
                                    func=Ident,
                                    bias=bias_t[:, dc:dc + 1],
                                    scale=scale,
                                )
                # v in normal [l, d] layout
                wv_s = load_w(wvT, "wv")
                with tc.tile_pool(name="x_v", bufs=1) as xr, \
                     tc.tile_pool(name="ps_v", bufs=4, space="PSUM") as pp:
                    xts = [load_x(xr, xvT, ec) for ec in range(ne)]
                    for lt in range(nk):
                        ps = pp.tile([128, DG], f32, tag="pv", name="pv")
                        for ec in range(ne):
                            nc.tensor.matmul(
                                out=ps,
                                lhsT=xts[ec][:, lt * 128:(lt + 1) * 128],
                                rhs=wv_s[:, ec, :],
                                start=(ec == 0), stop=(ec == ne - 1),
                            )
                        nc.scalar.activation(
                            out=v_s[:, lt, :, 0:Dh],
                            in_=ps.rearrange("p (h d) -> p h d", h=HPC),
                            func=Copy,
                        )

            # ---------------- attention + output projection ----------------
            with tc.tile_pool(name="stp", bufs=1) as stp:
                st_s = stp.tile([Dh, HPC, nq, 512], cdt, tag="st", name="st")
                with tc.tile_pool(name="ps_att", bufs=2, space="PSUM") as sp, \
                     tc.tile_pool(name="ps_out", bufs=1, space="PSUM") as op, \
                     tc.tile_pool(name="pt", bufs=6) as ptp, \
                     tc.tile_pool(name="mk", bufs=3) as mkp, \
                     tc.tile_pool(name="nrm", bufs=4) as nrm, \
                     tc.tile_pool(name="drp", bufs=4, space="DRAM") as drp:
                    for qn in range(nq):
                        kmax = min(nk, 4 * qn + 4) if variant == "causal" else nk
                        ps_o = [op.tile([Dv, 512], f32, tag=f"po{h}",
                                        name=f"po{h}") for h in range(HPC)]
                        for kc in range(kmax):
                            if variant == "general":
                                mkt = mkp.tile([128, 512], f32, tag="mkt",
                                               name="mkt")
                                nc.sync.dma_start(
                                    out=mkt,
                                    in_=maskT[kc * 128:(kc + 1) * 128,
                                              qn * 512:(qn + 1) * 512])
                            dblk = kc - 4 * qn
                            for h in range(HPC):
                                pb = (h % 2) * 64
                                dc = h // 2
                                ps_s = sp.tile([128, 512], f32, tag="pss",
                                               name="pss")
                                nc.tensor.matmul(
                                    out=ps_s,
                                    lhsT=kT_s[pb:pb + 64, dc,
                                              kc * 128:(kc + 1) * 128],
                                    rhs=qT_s[pb:pb + 64, dc,
                                             qn * 512:(qn + 1) * 512],
                                    start=True, stop=True,
                                )
                                if variant == "general":
                                    nc.vector.tensor_add(out=ps_s, in0=ps_s,
                                                         in1=mkt)
                                elif variant == "causal" and 0 <= dblk <= 3:
                                    nc.vector.tensor_add(out=ps_s, in0=ps_s,
                                                         in1=dm_s[:, dblk, :])
                                pt = ptp.tile([128, 512], cdt, tag="pt",
                                              name="pt")
                                nc.scalar.activation(out=pt, in_=ps_s, func=Exp)
                                nc.tensor.matmul(
                                    out=ps_o[h],
                                    lhsT=v_s[:, kc, h, :],
                                    rhs=pt,
                                    start=(kc == 0), stop=(kc == kmax - 1),
                                )
                        for h in range(HPC):
                            rec = nrm.tile([128, 512], f32, tag="rec",
                                           name="rec")
                            nc.vector.reciprocal(out=rec[Dh:Dh + 1, :],
                                                 in_=ps_o[h][Dh:Dh + 1, :])
                            dscr = drp.tile([1, 512], f32, tag="dscr",
                                            name="dscr")
                            nc.sync.dma_start(out=dscr, in_=rec[Dh:Dh + 1, :])
                            rb = nrm.tile([Dh, 512], f32, tag="rb", name="rb")
                            nc.sync.dma_start(
                                out=rb, in_=dscr.to_broadcast([Dh, 512]))
                            nc.vector.tensor_mul(out=st_s[:, h, qn, :],
                                                 in0=ps_o[h][0:Dh, :], in1=rb)

                with tc.tile_pool(name="ps_y", bufs=4, space="PSUM") as yp, \
                     tc.tile_pool(name="yst", bufs=3) as ys:
                    for jc in range(ne):
                        for ln in range(nq):
                            yps = yp.tile([128, 512], f32, tag="yps",
                                          name="yps")
                            for h in range(HPC):
                                nc.tensor.matmul(
                                    out=yps,
                                    lhsT=wo_s[:, h, jc * 128:(jc + 1) * 128],
                                    rhs=st_s[:, h, ln, :],
                                    start=(h == 0), stop=(h == HPC - 1),
                                )
                            yt = ys.tile([128, 512], f32, tag="yt", name="yt")
                            nc.scalar.copy(out=yt, in_=yps)
                            nc.sync.dma_start(
                                out=yT[jc * 128:(jc + 1) * 128,
                                       ln * 512:(ln + 1) * 512],
                                in_=yt)

    nc.finalize()
    return nc


def _get_nc(variant, Lb=L, reps=1):
    key = (variant, Lb, USE_F32R, reps)
    if key not in _CACHE:
        _CACHE[key] = _build(variant, Lb, USE_F32R, reps)
    return _CACHE[key]


def _detect_variant(mask):
    m2 = np.asarray(mask).reshape(mask.shape[-2], mask.shape[-1])
    m01 = (m2 != 0)
    if m01.all():
        return "none", m2
    if np.array_equal(m01, np.tril(np.ones(m2.shape, bool))):
        return "causal", m2
    return "general", m2


def _dmask_np():
    kl = np.arange(128)[:, None, None]
    db = np.arange(4)[None, :, None]
    ql = np.arange(512)[None, None, :]
    return np.where(db * 128 + kl > ql, NEG, np.float32(0)).astype(np.float32)


def _make_in_maps(x_q, x_k, x_v, m2, variant, Wq, bq, Wk, bk, Wv, Wo):
    in_maps = []
    for c in range(NCORES):
        b, g = divmod(c, HPC)
        gs = slice(g * DG, (g + 1) * DG)
        im = {
            "xqT": np.ascontiguousarray(x_q[b].T),
            "xkT": np.ascontiguousarray(x_k[b].T),
            "xvT": np.ascontiguousarray(x_v[b].T),
            "wqT": np.ascontiguousarray(Wq[gs, :].T),
            "wkT": np.ascontiguousarray(Wk[gs, :].T),
            "wvT": np.ascontiguousarray(Wv[gs, :].T),
            "bqs": np.ascontiguousarray((bq[gs] / 8.0).reshape(2, 128).T),
            "bks": np.ascontiguousarray(bk[gs].reshape(2, 128).T),
            "woT": np.ascontiguousarray(
                Wo[:, gs].T.reshape(HPC, Dh, E).transpose(1, 0, 2)),
        }
        if variant == "causal":
            im["dmask"] = _dmask_np()
        elif variant == "general":
            madd = np.where(m2 == 0, NEG, np.float32(0)).astype(np.float32)
            im["maskT"] = np.ascontiguousarray(madd.T)
        in_maps.append(im)
    return in_maps


def kernel(x_q, x_k, x_v, mask, Wq, bq, Wk, bk, Wv, bv, Wo, bo):
    global LAST_RESULTS
    from concourse.bass_utils import run_bass_kernel_spmd

    x_q = np.asarray(x_q, np.float32)
    x_k = np.asarray(x_k, np.float32)
    x_v = np.asarray(x_v, np.float32)
    Wq = np.asarray(Wq, np.float32)
    Wk = np.asarray(Wk, np.float32)
    Wv = np.asarray(Wv, np.float32)
    Wo = np.asarray(Wo, np.float32)
    bq = np.asarray(bq, np.float32)
    bk = np.asarray(bk, np.float32)
    bv = np.asarray(bv, np.float32)
    bo = np.asarray(bo, np.float32)

    variant, m2 = _detect_variant(mask)
    nc = _get_nc(variant)
    in_maps = _make_in_maps(x_q, x_k, x_v, m2, variant, Wq, bq, Wk, bk, Wv, Wo)

    res = run_bass_kernel_spmd(nc, in_maps, core_ids=list(range(NCORES)))
    LAST_RESULTS = res

    corr = (bo + Wo @ bv).astype(np.float32)
    y = np.empty((B, L, E), np.float32)
    for b in range(B):
        acc = res.results[HPC * b]["yT"].copy()
        for g in range(1, HPC):
            acc += res.results[HPC * b + g]["yT"]
        y[b] = acc.T + corr
    return y
